# revision 8
# baseline (speedup 1.0000x reference)
"""Trainium2 Bass kernel for nn_BatchCropElements: out = x * (rand_u > 0.3).

Full inputs: x [64, 2048, 24, 12] f32, rand_u [24, 12] f32.
Sharding: data-parallel on batch across 8 cores -> per-core x [8, 2048, 24, 12],
viewed flat as [128 partitions, 36864 free] f32 (36864 = 128 spatial planes of
288 = 24*12 elements, so the mask pattern tiles the free dim exactly).
rand_u is replicated to every core (host pre-broadcasts it to [128, 288]).

Per core: build the f32 0/1 mask once in SBUF (threshold + log-doubling widen
to one chunk width), then stream 8 chunks of [128, 4608]: HWDGE load (sync) ->
DVE tensor-tensor multiply -> HWDGE store (scalar). Memory-bound; DMA overlap
via tile pools.
"""

import numpy as np

import concourse.bass as bass
import concourse.tile as tile
from concourse import bacc, mybir
from concourse.bass_utils import run_bass_kernel_spmd

N_CORES = 8
B, C, H, W = 64, 2048, 24, 12
HW = H * W  # 288
B_SH = B // N_CORES  # 8 batches per core
P = 128
F_TOTAL = B_SH * C * HW // P  # 36864 f32 per partition
F = 4608  # chunk free size (16 spatial planes); F % HW == 0
N_CHUNK = F_TOTAL // F  # 8
PROB = 0.3

_DT = mybir.dt.float32


def _build_nc() -> bass.Bass:
    # Bacc (not raw Bass): its finalize pipeline splits multi-wait sync into
    # event-semaphore chains — TRN2 allows at most 1 wait per instruction.
    nc = bacc.Bacc()
    x = nc.declare_dram_parameter("x", [P, F_TOTAL], _DT, isOutput=False)
    u = nc.declare_dram_parameter("u", [P, HW], _DT, isOutput=False)
    out = nc.declare_dram_parameter("out", [P, F_TOTAL], _DT, isOutput=True)

    # The walrus TensorTensor encoding fits only one embedded sync wait, so
    # the structure keeps every DVE op at <=1 wait: bufs == N_CHUNK (no SBUF
    # slot reuse -> loads need no WAR wait, muls wait only on their own
    # load), in-place multiply, and a 1-element "absorber" copy that soaks up
    # the mask-ready wait so the first mul doesn't carry two.
    with tile.TileContext(nc) as tc:
        with (
            tc.tile_pool(name="upool", bufs=1) as upool,
            tc.tile_pool(name="maskp", bufs=1) as maskp,
            tc.tile_pool(name="scrp", bufs=1) as scrp,
            tc.tile_pool(name="iop", bufs=N_CHUNK // 2) as iop,
        ):
            tu = upool.tile([P, HW], _DT)
            nc.sync.dma_start(out=tu[:], in_=u[:, :])
            bmask = maskp.tile([P, F], _DT)
            nc.vector.tensor_scalar(
                out=bmask[:, 0:HW],
                in0=tu[:],
                scalar1=PROB,
                scalar2=None,
                op0=mybir.AluOpType.is_gt,
            )
            w = HW
            while w < F:
                nc.vector.tensor_copy(out=bmask[:, w : 2 * w], in_=bmask[:, 0:w])
                w *= 2
            scr = scrp.tile([1, 1], _DT)
            nc.vector.tensor_copy(out=scr[:], in_=bmask[0:1, F - 1 : F])

            # Paired chunks: 8 loads on the HWDGE lanes, but one SWDGE store
            # per [P, 2F] double-tile so only 4 DMA-SW lanes are used. Keeps
            # the kernel-tail drain's sem-wait list (1 DVE + 8 HW + 4 SW)
            # under the CTRL struct's capacity, stores at 1 wait (DVE), and
            # stores off the HWDGE lanes where reuse would add a second wait.
            for k in range(N_CHUNK // 2):
                t = iop.tile([P, 2 * F], _DT, name="t")
                for h in range(2):
                    c = 2 * k + h
                    sl = slice(h * F, (h + 1) * F)
                    nc.sync.dma_start(out=t[:, sl], in_=x[:, c * F : (c + 1) * F])
                    nc.vector.tensor_mul(out=t[:, sl], in0=t[:, sl], in1=bmask[:])
                nc.gpsimd.dma_start(
                    out=out[:, 2 * k * F : 2 * (k + 1) * F], in_=t[:]
                )
    nc.finalize()
    return nc


_NC_CACHE: list = []


def _run(inputs: dict, trace: bool = False):
    x = np.ascontiguousarray(inputs["x"], dtype=np.float32)
    rand_u = np.ascontiguousarray(inputs["rand_u"], dtype=np.float32)
    assert x.shape == (B, C, H, W), x.shape
    assert rand_u.shape == (H, W), rand_u.shape

    u_rep = np.ascontiguousarray(
        np.broadcast_to(rand_u.reshape(1, HW), (P, HW)), dtype=np.float32
    )
    in_maps = []
    for i in range(N_CORES):
        shard = x[i * B_SH : (i + 1) * B_SH].reshape(P, F_TOTAL)
        in_maps.append({"x": shard, "u": u_rep})

    if not _NC_CACHE:
        _NC_CACHE.append(_build_nc())
    nc = _NC_CACHE[0]

    res = run_bass_kernel_spmd(nc, in_maps, list(range(N_CORES)), trace=trace)
    out = np.empty((B, C, H, W), dtype=np.float32)
    for i in range(N_CORES):
        out[i * B_SH : (i + 1) * B_SH] = res.results[i]["out"].reshape(
            B_SH, C, H, W
        )
    return out, res


def kernel(**inputs: np.ndarray) -> np.ndarray:
    out, _ = _run(inputs, trace=False)
    return out
